# revision 9
# baseline (speedup 1.0000x reference)
"""Trainium2 Bass kernel for nn_CustomMetalPKA_GNN (gnn_message_passing).

Distribution: node-sharded GCN message passing across 8 NeuronCores.
Each core owns a contiguous block of 1280 node rows (10 windows of 128).
Edges are assigned to the core owning their destination node, sorted by
destination, and processed as 128-edge tiles:
  gather(src rows via dma_gather) -> scaled-one-hot scatter-matmul into PSUM.

The symmetric GCN norm dinv[s]*dinv[d] is fully host-factorized:
  - dinv[s] is folded into the gathered table rows (x_pre is pre-scaled on
    host; layer-2 table rows are scaled before the AllGather), and
  - dinv[d] is folded into the one-hot matrix entries (host-computed
    per-edge `nrm`, applied in the same DVE op that builds the one-hot).

Scatter matmuls are operand-swapped (lhsT = gathered rows, rhs = one-hot)
so they directly produce feature-major (transposed) activations; the
dense-layer matmuls then consume those as lhsT without any PE transposes.
Epilogues (bias + relu + bf16 cast) run as single Activation-engine ops
with per-partition bias (features sit on partitions in the transposed
layout). All matmul operands are bf16 (1 PE cycle/row vs 4 for fp32).

Layer boundary: one AllGather of the per-core layer-1 table rows. The
tiny metal/transformer tail is reduced to an [8, 512] summary (3
ligand-block sums + 3 prediction rows) via a mask matmul and finished on
host (0.006% of FLOPs).
"""

import os
import sys

for _p in ("/opt/trn_rl_repo", "/root/.axon_site/_ro/trn_rl_repo"):
    if os.path.isdir(_p) and _p not in sys.path:
        sys.path.insert(0, _p)

import numpy as np

import concourse.bacc as bacc
import concourse.tile as tile
from concourse import bass, mybir
from concourse.bass_utils import run_bass_kernel_spmd

# Problem shapes (hardcoded per spec)
N = 9999
E = 160000
NODE_D = 256
HID = 512
MAX_LIG = 3
APL = N // MAX_LIG  # 3333

NCORES = 8
P = 128
WPC = 10                 # windows per core
NPC = WPC * P            # 1280 nodes per core
NPAD = NCORES * NPC      # 10240
NW = NCORES * WPC        # 80 global windows

FP = mybir.dt.float32
BF = mybir.dt.bfloat16
I16 = mybir.dt.int16
F8 = mybir.dt.float8e4
GCHUNK = int(os.environ.get("KGCHUNK", "6"))   # gather tiles per dma_gather call
ABL_NOGATHER = os.environ.get("KABL_NOGATHER", "") == "1"
ABL_NOCC = os.environ.get("KABL_NOCC", "") == "1"
ABL_NOMM = os.environ.get("KABL_NOMM", "") == "1"
ABL_TINYCC = os.environ.get("KABL_TINYCC", "") == "1"
DMA_SCRATCH = int(os.environ.get("KSCRATCH", "16384"))

_RUN_CACHE = {}


# ----------------------------------------------------------------------------
# Host-side graph preprocessing (index/structure + elementwise input prep)
# ----------------------------------------------------------------------------

def _prep(x, edge_index, pred_pos):
    src = np.asarray(edge_index[0], dtype=np.int64)
    dst = np.asarray(edge_index[1], dtype=np.int64)
    pred_pos = np.asarray(pred_pos, dtype=np.int64)

    deg = np.bincount(dst, minlength=N).astype(np.float32) + 1.0
    dinv = deg ** -0.5
    dinv_pad = np.ones(NPAD, np.float32)
    dinv_pad[:N] = dinv

    order = np.argsort(dst, kind="stable")
    s_s = src[order]
    d_s = dst[order]

    # per (core, window) edge slices
    bounds = np.empty((NCORES, WPC, 2), np.int64)
    for c in range(NCORES):
        for w in range(WPC):
            lo = c * NPC + w * P
            hi = min(lo + P, N)
            if lo >= N:
                lo = hi = N  # empty
            bounds[c, w, 0] = np.searchsorted(d_s, lo, side="left")
            bounds[c, w, 1] = np.searchsorted(d_s, hi, side="left")

    # tiles per window: edges + self-loops (for real nodes), padded to 128
    T = []
    for w in range(WPC):
        mx = 1
        for c in range(NCORES):
            lo = c * NPC + w * P
            nself = max(0, min(lo + P, N) - lo)
            cnt = int(bounds[c, w, 1] - bounds[c, w, 0]) + nself
            mx = max(mx, (cnt + P - 1) // P)
        T.append(mx)
    Tsum = sum(T)
    CTOT = 8 * Tsum  # int16 index columns

    # per-core arrays
    per_core = []
    for c in range(NCORES):
        gidx = np.zeros((P, CTOT), np.int16)
        gidx2 = np.zeros((P, CTOT), np.int16)
        doff = np.full((P, Tsum), -1.0, np.float32)
        dnrm = np.zeros((P, Tsum), np.float32)
        off = 0
        for w in range(WPC):
            lo, hi = int(bounds[c, w, 0]), int(bounds[c, w, 1])
            base = c * NPC + w * P
            nself = max(0, min(base + P, N) - base)
            e_src = np.concatenate([s_s[lo:hi], np.arange(base, base + nself)])
            e_dst = np.concatenate([d_s[lo:hi], np.arange(base, base + nself)])
            e_off = np.concatenate([d_s[lo:hi] - base, np.arange(nself)])
            n = e_src.shape[0]
            cap = T[w] * P
            srcs = np.zeros(cap, np.int64)
            offs = np.full(cap, -1.0, np.float32)
            nrms = np.zeros(cap, np.float32)
            srcs[:n] = e_src
            offs[:n] = e_off.astype(np.float32)
            nrms[:n] = dinv[e_dst]
            # pack indices: entry i -> gidx[i % 16, colbase + i // 16]
            # (replicated to all 8 Q7-core stripes of 16 partitions below)
            colbase = 8 * off
            ii = np.arange(cap)
            gidx[ii % 16, colbase + ii // 16] = srcs.astype(np.int16)
            # w-major physical index for the chunked-AllGather table2 layout:
            # node (c_s, w_s, p_s) lives at w_s*1024 + c_s*128 + p_s
            phys = ((srcs % NPC) // P) * (NCORES * P) + (srcs // NPC) * P + (srcs % P)
            gidx2[ii % 16, colbase + ii // 16] = phys.astype(np.int16)
            doff[:, off:off + T[w]] = offs.reshape(T[w], P).T
            dnrm[:, off:off + T[w]] = nrms.reshape(T[w], P).T
            off += T[w]
        gidx[16:] = np.tile(gidx[:16], (7, 1))
        gidx2[16:] = np.tile(gidx2[:16], (7, 1))

        # own-window dinv [P, WPC] for the table2 row scale
        dinvT = dinv_pad[c * NPC:(c + 1) * NPC].reshape(WPC, P).T.copy()

        # tail masks [P, 8 * WPC]
        tmask = np.zeros((P, 8 * WPC), np.float32)
        for w in range(WPC):
            base = c * NPC + w * P
            nodes = base + np.arange(P)
            real = nodes < N
            for b in range(MAX_LIG):
                sel = real & (nodes >= b * APL) & (nodes < (b + 1) * APL)
                tmask[sel, 8 * w + b] = 1.0
            for i in range(MAX_LIG):
                sel = nodes == pred_pos[i]
                tmask[sel, 8 * w + 3 + i] = 1.0
        import ml_dtypes
        per_core.append(dict(gidx=gidx, gidx2=gidx2, doff=doff, dnrm=dnrm, dinvT=dinvT,
                             tmask=tmask.astype(ml_dtypes.bfloat16)))

    # pre-scaled bf16 node features (dinv[src] folded in)
    import ml_dtypes
    x_pre = np.zeros((NPAD, NODE_D), ml_dtypes.bfloat16)
    x_pre[:N] = (np.asarray(x, np.float32) * dinv[:, None]).astype(ml_dtypes.bfloat16)

    iota = np.tile(np.arange(P, dtype=np.float32)[None, :], (P, 1))

    meta = dict(T=T, Tsum=Tsum, CTOT=CTOT)
    return meta, per_core, x_pre, iota


def prepare(inputs):
    """Build (meta, in_maps) for the device program from full inputs."""
    import ml_dtypes
    x = np.asarray(inputs["x"], np.float32)
    edge_index = np.asarray(inputs["edge_index"])
    pred_pos = np.asarray(inputs["pred_pos"])

    meta, per_core, x_pre, iota = _prep(x, edge_index, pred_pos)

    bf = ml_dtypes.bfloat16
    g1_w = np.asarray(inputs["g1_w"], np.float32).astype(bf)
    g2_w = np.asarray(inputs["g2_w"], np.float32).astype(bf)
    lp_w = np.asarray(inputs["lp_w"], np.float32).astype(bf)
    # transposed per-partition biases: bT[p, c] = b[c*128 + p]
    b1T = np.asarray(inputs["g1_b"], np.float32).reshape(HID // P, P).T.copy()
    b2T = np.asarray(inputs["g2_b"], np.float32).reshape(HID // P, P).T.copy()
    blp_rep = np.tile(np.asarray(inputs["lp_b"], np.float32)[None, :], (P, 1))

    in_maps = []
    for c in range(NCORES):
        pc = per_core[c]
        in_maps.append(dict(
            x_pre=x_pre,
            gidx=pc["gidx"],
            gidx2=pc["gidx2"],
            doff=pc["doff"],
            dnrm=pc["dnrm"],
            dinvT=pc["dinvT"],
            tmask=pc["tmask"],
            iota=iota,
            g1_w=g1_w, g2_w=g2_w, lp_w=lp_w,
            b1T=b1T, b2T=b2T, blp_rep=blp_rep,
        ))
    return meta, in_maps


# ----------------------------------------------------------------------------
# Device program
# ----------------------------------------------------------------------------

def _build(meta, sim1=False, reps=1):
    T = meta["T"]
    Tsum = meta["Tsum"]
    CTOT = meta["CTOT"]
    KD = NODE_D // P   # 2 feature chunks in layer-1 scatter
    KH = HID // P      # 4 feature chunks elsewhere

    nc = bacc.Bacc("TRN2", target_bir_lowering=False, debug=False,
                   num_devices=1 if sim1 else NCORES,
                   dynamic_dma_scratch_size=DMA_SCRATCH)

    # inputs
    d_x = nc.declare_dram_parameter("x_pre", [NPAD, NODE_D], BF, isOutput=False)
    d_gidx = nc.declare_dram_parameter("gidx", [P, CTOT], I16, isOutput=False)
    d_gidx2 = nc.declare_dram_parameter("gidx2", [P, CTOT], I16, isOutput=False)
    d_doff = nc.declare_dram_parameter("doff", [P, Tsum], FP, isOutput=False)
    d_dnrm = nc.declare_dram_parameter("dnrm", [P, Tsum], FP, isOutput=False)
    d_dinvT = nc.declare_dram_parameter("dinvT", [P, WPC], FP, isOutput=False)
    d_tmask = nc.declare_dram_parameter("tmask", [P, 8 * WPC], BF, isOutput=False)
    d_iota = nc.declare_dram_parameter("iota", [P, P], FP, isOutput=False)
    d_g1 = nc.declare_dram_parameter("g1_w", [NODE_D, HID], BF, isOutput=False)
    d_g2 = nc.declare_dram_parameter("g2_w", [HID, HID], BF, isOutput=False)
    d_lp = nc.declare_dram_parameter("lp_w", [HID, HID], BF, isOutput=False)
    d_b1T = nc.declare_dram_parameter("b1T", [P, KH], FP, isOutput=False)
    d_b2T = nc.declare_dram_parameter("b2T", [P, KH], FP, isOutput=False)
    d_blp = nc.declare_dram_parameter("blp_rep", [P, HID], FP, isOutput=False)
    # output
    d_tail = nc.declare_dram_parameter("out_tail", [8, HID], FP, isOutput=True)

    # internal dram
    local_rows = nc.dram_tensor("local_rows", [NPC, HID], F8)
    table2 = nc.dram_tensor("table2", [NPAD, HID], F8, addr_space="Shared")
    tiny_in = nc.dram_tensor("tiny_in", [8, HID], BF)
    tiny_out = nc.dram_tensor("tiny_out", [8 * NCORES, HID], BF, addr_space="Shared")

    mm = mybir.AluOpType
    act = mybir.ActivationFunctionType

    with tile.TileContext(nc) as tc:
        with (
            tc.tile_pool(name="const", bufs=1) as cpool,
            tc.tile_pool(name="work", bufs=3) as wpool,
            tc.tile_pool(name="hT", bufs=2) as hTpool,
            tc.tile_pool(name="ps_sc", bufs=2, space="PSUM") as ps_sc,
            tc.tile_pool(name="ps_mm", bufs=2, space="PSUM") as ps_mm,
            tc.tile_pool(name="ps_t2", bufs=2, space="PSUM") as ps_t2,
            tc.tile_pool(name="ps_tail", bufs=1, space="PSUM") as ps_tail,
        ):
            # ---- constants in ----
            t_idx = cpool.tile([P, CTOT], I16)
            nc.sync.dma_start(t_idx[:], d_gidx[:])
            t_idx2 = cpool.tile([P, CTOT], I16)
            nc.sync.dma_start(t_idx2[:], d_gidx2[:])
            t_doff = cpool.tile([P, Tsum], FP)
            nc.sync.dma_start(t_doff[:], d_doff[:])
            t_dnrm = cpool.tile([P, Tsum], FP)
            nc.sync.dma_start(t_dnrm[:], d_dnrm[:])
            t_iota = cpool.tile([P, P], FP)
            nc.sync.dma_start(t_iota[:], d_iota[:])
            t_tmask = cpool.tile([P, 8 * WPC], BF)
            nc.sync.dma_start(t_tmask[:], d_tmask[:])
            t_g1 = cpool.tile([P, KD, HID], BF)
            nc.sync.dma_start(t_g1[:], d_g1.rearrange("(a p) n -> p a n", p=P))
            t_g2 = cpool.tile([P, KH, HID], BF)
            nc.sync.dma_start(t_g2[:], d_g2.rearrange("(a p) n -> p a n", p=P))
            t_lp = cpool.tile([P, KH, HID], BF)
            nc.sync.dma_start(t_lp[:], d_lp.rearrange("(a p) n -> p a n", p=P))
            t_b1T = cpool.tile([P, KH], FP)
            nc.sync.dma_start(t_b1T[:], d_b1T[:])
            t_b2T = cpool.tile([P, KH], FP)
            nc.sync.dma_start(t_b2T[:], d_b2T[:])
            t_blp = cpool.tile([P, HID], FP)
            nc.sync.dma_start(t_blp[:], d_blp[:])
            t_dinv = cpool.tile([P, WPC], FP)
            nc.sync.dma_start(t_dinv[:], d_dinvT[:])

            t_ohc = cpool.tile([P, Tsum, P], BF)

            def onehot_build(col):
                # oh[e, j] = (iota[j] == doff[e]) * dinv_dst[e]
                nc.vector.tensor_scalar(
                    out=t_ohc[:, col, :],
                    in0=t_iota[:],
                    scalar1=t_doff[:, col:col + 1],
                    scalar2=t_dnrm[:, col:col + 1],
                    op0=mm.is_equal,
                    op1=mm.mult)
                return t_ohc[:, col, :]

            for rep_i in range(reps):
                # ---- layer 1 ----
                off_w = 0
                g1ctx = tc.tile_pool(name="g1pool", bufs=2)
                g1pool = g1ctx.__enter__()
                for w in range(WPC):
                    Tw = T[w]
                    g1t = g1pool.tile([P, Tw, NODE_D], BF, tag="gather1")
                    for t0 in ([] if ABL_NOGATHER else range(0, Tw, GCHUNK)):
                        t1 = min(t0 + GCHUNK, Tw)
                        nc.gpsimd.dma_gather(
                            out_ap=g1t[:, t0:t1, :],
                            in_ap=d_x[:],
                            idxs_ap=t_idx[:, 8 * (off_w + t0): 8 * (off_w + t1)],
                            num_idxs=(t1 - t0) * P,
                            num_idxs_reg=(t1 - t0) * P,
                            elem_size=NODE_D)
                    # scatter (swapped): psxT[f, n] += g1t[e, f]^T @ oh[e, n]
                    # one full PSUM bank; slice k owns columns [k*P, (k+1)*P).
                    # start=True zeroes the whole 2KB zero region, so only the
                    # first matmul into the bank starts the group.
                    psxT = ps_sc.tile([P, HID], FP, tag="psc")
                    for t in ([] if ABL_NOMM else range(Tw)):
                        oh = onehot_build(off_w + t)
                        for k in range(KD):
                            nc.tensor.matmul(psxT[:, k * P:(k + 1) * P],
                                             g1t[:, t, k * P:(k + 1) * P], oh,
                                             start=(t == 0 and k == 0),
                                             stop=(t == Tw - 1 and k == KD - 1))
                    sxT = hTpool.tile([P, KD, P], BF, tag="sxT")
                    for k in range(KD):
                        nc.scalar.copy(sxT[:, k, :], psxT[:, k * P:(k + 1) * P])
                    # XW1 (transposed out): hT[f2, n] = sum_k W1[k][f2]^T... via
                    # lhsT = g1 chunk [f, f2], rhs = sxT[f, n]
                    psh = ps_mm.tile([P, HID], FP, tag="pmm")
                    for c2 in range(KH):
                        for k in range(KD):
                            nc.tensor.matmul(
                                psh[:, c2 * P:(c2 + 1) * P],
                                t_g1[:, k, c2 * P:(c2 + 1) * P], sxT[:, k, :],
                                start=(c2 == 0 and k == 0),
                                stop=(c2 == KH - 1 and k == KD - 1))
                    hT = hTpool.tile([P, KH, P], BF, tag="hT")
                    for c2 in range(KH):
                        nc.scalar.activation(hT[:, c2, :],
                                             psh[:, c2 * P:(c2 + 1) * P], act.Relu,
                                             bias=t_b1T[:, c2:c2 + 1])
                    # table2 rows: t2[n, :] = dinv_own * (h @ W2)
                    pst2 = ps_t2.tile([P, HID], FP, tag="pt2")
                    for c2 in range(KH):
                        nc.tensor.matmul(pst2[:], hT[:, c2, :], t_g2[:, c2, :],
                                         start=(c2 == 0), stop=(c2 == KH - 1))
                    t2 = wpool.tile([P, HID], F8, tag="t2")
                    nc.scalar.activation(t2[:], pst2[:], act.Copy,
                                         scale=t_dinv[:, w:w + 1])
                    nc.sync.dma_start(local_rows[w * P:(w + 1) * P, :], t2[:])
                    if not (sim1 or ABL_NOCC or ABL_TINYCC):
                        # chunked AllGather: window w of all cores lands at
                        # table2[w*1024 : (w+1)*1024] (w-major layout),
                        # overlapping the transfer with later L1 windows.
                        nc.gpsimd.collective_compute(
                            "AllGather", mm.bypass,
                            replica_groups=[list(range(NCORES))],
                            ins=[local_rows[w * P:(w + 1) * P, :]],
                            outs=[table2[w * NCORES * P:(w + 1) * NCORES * P, :]])
                    off_w += Tw

                g1ctx.__exit__(None, None, None)

                # ---- exchange ----
                if sim1 or ABL_NOCC or ABL_TINYCC:
                    # no-collective fallback: stage own windows locally so the
                    # single-core timeline sim / ablations still run.
                    for g in range(WPC):
                        tt = wpool.tile([P, HID], F8, tag="t2")
                        nc.sync.dma_start(tt[:], local_rows[g * P:(g + 1) * P, :])
                        nc.sync.dma_start(table2[g * P:(g + 1) * P, :], tt[:])
                    if ABL_TINYCC:
                        nc.gpsimd.collective_compute(
                            "AllGather", mm.bypass,
                            replica_groups=[list(range(NCORES))],
                            ins=[local_rows[0:8, :]],
                            outs=[tiny_out[:]])

                # ---- layer 2 + proj + tail ----
                ptail = ps_tail.tile([8, HID], FP)
                off_w = 0
                g2ctx = tc.tile_pool(name="g2pool", bufs=2)
                g2pool = g2ctx.__enter__()
                for w in range(WPC):
                    Tw = T[w]
                    g2t = g2pool.tile([P, Tw, HID], F8, tag="gather2")
                    for t0 in ([] if ABL_NOGATHER else range(0, Tw, GCHUNK)):
                        t1 = min(t0 + GCHUNK, Tw)
                        nc.gpsimd.dma_gather(
                            out_ap=g2t[:, t0:t1, :],
                            in_ap=table2[:],
                            idxs_ap=t_idx2[:, 8 * (off_w + t0): 8 * (off_w + t1)],
                            num_idxs=(t1 - t0) * P,
                            num_idxs_reg=(t1 - t0) * P,
                            elem_size=HID)
                    ps2T = ps_sc.tile([P, HID], FP, tag="psc")
                    for t in ([] if ABL_NOMM else range(Tw)):
                        oh = t_ohc[:, off_w + t, :]
                        for k in range(KH):
                            nc.tensor.matmul(ps2T[:, k * P:(k + 1) * P],
                                             g2t[:, t, k * P:(k + 1) * P], oh,
                                             start=(t == 0 and k == 0),
                                             stop=(t == Tw - 1 and k == KH - 1))
                    h2T = hTpool.tile([P, KH, P], BF, tag="hT")
                    for k in range(KH):
                        nc.scalar.activation(h2T[:, k, :],
                                             ps2T[:, k * P:(k + 1) * P], act.Relu,
                                             bias=t_b2T[:, k:k + 1])
                    # h1 = relu(h2 @ lp + blp)
                    psh1 = ps_t2.tile([P, HID], FP, tag="pt2")
                    for k in range(KH):
                        nc.tensor.matmul(psh1[:], h2T[:, k, :], t_lp[:, k, :],
                                         start=(k == 0), stop=(k == KH - 1))
                    h1p = wpool.tile([P, HID], FP, tag="h1p")
                    nc.vector.tensor_add(h1p[:], psh1[:], t_blp[:])
                    h1 = wpool.tile([P, HID], BF, tag="h1")
                    nc.scalar.activation(h1[:], h1p[:], act.Relu)
                    nc.tensor.matmul(ptail[:], t_tmask[:, 8 * w:8 * w + 8], h1[:],
                                     start=(w == 0), stop=(w == WPC - 1))
                    off_w += Tw

                g2ctx.__exit__(None, None, None)

                t_tail = wpool.tile([8, HID], FP, tag="tailout")
                nc.vector.tensor_copy(t_tail[:], ptail[:])
                nc.sync.dma_start(d_tail[:], t_tail[:])

    nc.compile()
    return nc


# ----------------------------------------------------------------------------
# Host tail (metal branch + gates + 4-node TransformerConv + MLP head)
# ----------------------------------------------------------------------------

def _host_tail(tail, pred_pos, metal_id, metal_emb_table, mp_w, mp_b,
               gate_w1, gate_b1, gate_w2, gate_b2,
               tq_w, tq_b, tk_w, tk_b, tv_w, tv_b, tskip_w, tskip_b,
               pr_w1, pr_b1, pr_w2, pr_b2):
    f = np.float32
    pred_pos = np.asarray(pred_pos, np.int64)
    blocksum = tail[:3].astype(f)
    predrow = tail[3:6].astype(f)
    HEADS, HD = 8, HID // 8

    backbones = []
    for i in range(MAX_LIG):
        b = int(pred_pos[i]) // APL
        backbones.append((blocksum[b] - predrow[i]) / f(APL - 1))

    metal_node = np.maximum(
        np.asarray(metal_emb_table, f)[np.asarray(metal_id, np.int64)] @
        np.asarray(mp_w, f) + np.asarray(mp_b, f), 0)

    def tconv(hm, es, ed):
        n = hm.shape[0]
        q = (hm @ np.asarray(tq_w, f) + np.asarray(tq_b, f)).reshape(n, HEADS, HD)
        k = (hm @ np.asarray(tk_w, f) + np.asarray(tk_b, f)).reshape(n, HEADS, HD)
        v = (hm @ np.asarray(tv_w, f) + np.asarray(tv_b, f)).reshape(n, HEADS, HD)
        kj = k[es]
        vj = v[es]
        alpha = (q[ed] * kj).sum(-1) / np.sqrt(f(HD))
        amax = np.full((n, HEADS), -np.inf, f)
        np.maximum.at(amax, ed, alpha)
        ae = np.exp(alpha - amax[ed])
        den = np.zeros((n, HEADS), f)
        np.add.at(den, ed, ae)
        att = ae / den[ed]
        out = np.zeros((n, HEADS, HD), f)
        np.add.at(out, ed, vj * att[:, :, None])
        return out.reshape(n, HID) + hm @ np.asarray(tskip_w, f) + np.asarray(tskip_b, f)

    preds = []
    for n_lig in range(MAX_LIG, 0, -1):
        rows = []
        for i in range(n_lig):
            hb = backbones[i]
            g = 1.0 / (1.0 + np.exp(-(np.tanh(hb @ np.asarray(gate_w1, f) +
                                              np.asarray(gate_b1, f)) @
                                      np.asarray(gate_w2, f) +
                                      np.asarray(gate_b2, f))))
            rows.append(predrow[i] + g[0] * hb)
        hm = np.concatenate([metal_node, np.stack(rows)], 0).astype(f)
        es, ed = [], []
        for l in range(1, n_lig + 1):
            es += [0, l]
            ed += [l, 0]
        h3 = tconv(hm, np.array(es), np.array(ed))
        V = h3.mean(0)
        preds.append((V @ np.asarray(pr_w1, f) + np.asarray(pr_b1, f)) @
                     np.asarray(pr_w2, f) + np.asarray(pr_b2, f))
    return np.concatenate(preds).astype(np.float32)


# ----------------------------------------------------------------------------
# Entry point
# ----------------------------------------------------------------------------

def kernel(**inputs):
    meta, in_maps = prepare(inputs)

    key = (meta["Tsum"], tuple(meta["T"]))
    nc = _RUN_CACHE.get(key)
    if nc is None:
        nc = _build(meta)
        _RUN_CACHE[key] = nc

    res = run_bass_kernel_spmd(nc, in_maps, list(range(NCORES)))
    tail = np.zeros((8, HID), np.float32)
    for c in range(NCORES):
        tail += res.results[c]["out_tail"]

    return _host_tail(
        tail, inputs["pred_pos"], inputs["metal_id"], inputs["metal_emb_table"],
        inputs["mp_w"], inputs["mp_b"],
        inputs["gate_w1"], inputs["gate_b1"], inputs["gate_w2"], inputs["gate_b2"],
        inputs["tq_w"], inputs["tq_b"], inputs["tk_w"], inputs["tk_b"],
        inputs["tv_w"], inputs["tv_b"], inputs["tskip_w"], inputs["tskip_b"],
        inputs["pr_w1"], inputs["pr_b1"], inputs["pr_w2"], inputs["pr_b2"])
